# revision 3
# baseline (speedup 1.0000x reference)
"""Chebyshev positional-embedding expansion kernel for Trainium2 (8 NeuronCores).

Computes out[b, s, d] = T_d(xhat[b, s]) for d = 0..D-1, where
xhat = 2*input_ids/max_seq_len - 1 and T_d is the d-th Chebyshev polynomial
of the first kind, matching the jax.lax.scan reference recurrence
    T_n = 2*xhat*T_{n-1} - T_{n-2}.

Strategy (per core; batch row b == core id, no communication):
  - 4096 seq positions per core laid out as [128 partitions, 32 h-slots],
    s_local = p*32 + h, so each partition owns a contiguous run of output
    rows -> fully contiguous per-partition DMA stores.
  - Seed block T_0..T_BASE computed with the exact sequential recurrence.
  - Remaining columns via Chebyshev block doubling
        T_{k+j} = 2*T_k*T_j - T_{k-j},  j = 1..k
    On the DVE this is ONE fused op (scalar_tensor_tensor) per (level, h):
        out = (block * 2T_k[per-partition scalar]) - reversed(block)
    The Pool engine (walrus rejects scalar_tensor_tensor there) uses
    tensor_scalar + tensor_tensor instead.
  - Work is split between the DVE (vector) and Pool (gpsimd) engines via
    disjoint h-slot pools held in separate SBUF tiles, so both engines run
    independent dependency chains in parallel.
  - h-slots are grouped into sub-tiles; each finished [128, nh*1024] tile
    is DMA'd to HBM (HWDGE) while later groups still compute, overlapping
    the ~47us/core of output DMA with the compute.
"""

import numpy as np

import concourse.bacc as bacc
import concourse.mybir as mybir
from concourse import tile
from concourse.bass_utils import run_bass_kernel_spmd

F32 = mybir.dt.float32
OP = mybir.AluOpType
AF = mybir.ActivationFunctionType

N_CORES = 8
B, S, D = 8, 4096, 1024
MAX_SEQ_LEN = 4096
S_PER = B * S // N_CORES  # 4096 seq positions per core
P = 128  # SBUF partitions
H = S_PER // P  # 32 h-slots per partition

# (dve_h, gp_h) per group; first group small so the first store DMA starts
# early and the output-DMA stream stays saturated from then on.
GROUPS = ((1, 1), (3, 1), (5, 1), (5, 1), (5, 1), (6, 1), (1, 0))


def _emit_body(nc, tc, pools, s_per, d, base, sub_tiles, seed_splits,
               max_seq_len, x2d, out3, dma_split, t2c_act):
    xpool, seedpool, gpool, spool = pools
    h = s_per // P

    levels = []
    k = base
    while k < d - 1:
        levels.append((k, min(k, d - 1 - k)))
        k *= 2

    X = xpool.tile([P, h], F32, tag="X")
    XHAT = xpool.tile([P, h], F32, tag="XHAT")
    X2 = xpool.tile([P, h], F32, tag="X2")
    nc.sync.dma_start(X[:], x2d)
    # xhat = x/2048 - 1  (bit-exact with 2*x/4096 - 1)
    nc.vector.tensor_scalar(XHAT[:], X[:], 1.0 / (max_seq_len / 2),
                            -1.0, OP.mult, OP.add)
    # 2*xhat = x/1024 - 2 (exact doubling of xhat)
    nc.vector.tensor_scalar(X2[:], X[:], 1.0 / (max_seq_len / 4),
                            -2.0, OP.mult, OP.add)

    # ---- seed blocks: Q[p, hh, n] = T_n(xhat[p, hh]), n = 0..base.
    # Two independent chains, one per engine, over disjoint h ranges.
    seeds = {}  # engine -> (Q tile, h_start, nh)
    for eng, h0, nh in seed_splits:
        if nh == 0:
            continue
        e = getattr(nc, eng)
        Q = seedpool.tile([P, nh, base + 1], F32, tag=f"Q_{eng}")
        PRD = seedpool.tile([P, nh], F32, tag=f"PRD_{eng}")
        e.memset(Q[:, :, 0], 1.0)
        e.tensor_copy(Q[:, :, 1], XHAT[:, h0:h0 + nh])
        for n in range(2, base + 1):
            e.tensor_tensor(PRD[:], X2[:, h0:h0 + nh], Q[:, :, n - 1],
                            OP.mult)
            e.tensor_tensor(Q[:, :, n], PRD[:], Q[:, :, n - 2], OP.subtract)
        seeds[eng] = (Q, h0, nh)

    # ---- per-sub-tile doubling + store
    dma_engines = [nc.sync, nc.scalar] if dma_split else [nc.sync]
    for ti, (eng, h0, nh) in enumerate(sub_tiles):
        e = getattr(nc, eng)
        Q, qh0, _ = seeds[eng]
        T = gpool.tile([P, nh, d], F32, tag="T")
        # seed columns 0..base for this tile's h-slots (ACT engine)
        nc.scalar.activation(T[:, :, 0:base + 1],
                             Q[:, h0 - qh0:h0 - qh0 + nh, :], AF.Copy)
        for (k, jmax) in levels:
            if eng == "vector":
                # fused one-op form: out = (block * 2T_k) - rev
                T2C = spool.tile([P, nh], F32, tag="T2C_v")
                if t2c_act:
                    nc.scalar.activation(T2C[:], T[:, :, k], AF.Copy,
                                         scale=2.0)
                else:
                    e.tensor_scalar(T2C[:], T[:, :, k], 2.0, None, OP.mult)
                for hh in range(nh):
                    rev = T[:, hh, k - 1:None if k - 1 - jmax < 0
                            else k - 1 - jmax:-1]
                    e.scalar_tensor_tensor(
                        T[:, hh, k + 1:k + 1 + jmax],
                        T[:, hh, 1:1 + jmax],
                        T2C[:, hh:hh + 1],
                        rev,
                        OP.mult, OP.subtract)
            else:
                # Pool path: tmp = (block * T_k) * 2, out = tmp - rev
                for hh in range(nh):
                    TMP = spool.tile([P, 512], F32, tag="TMP")
                    rev = T[:, hh, k - 1:None if k - 1 - jmax < 0
                            else k - 1 - jmax:-1]
                    e.tensor_scalar(TMP[:, 0:jmax], T[:, hh, 1:1 + jmax],
                                    T[:, hh, k:k + 1], 2.0, OP.mult, OP.mult)
                    e.tensor_tensor(T[:, hh, k + 1:k + 1 + jmax],
                                    TMP[:, 0:jmax], rev, OP.subtract)
        dma_engines[ti % len(dma_engines)].dma_start(
            out3[:, h0:h0 + nh, :], T[:])


def build_nc(s_per=S_PER, d=D, base=16, groups=GROUPS,
             max_seq_len=MAX_SEQ_LEN, gbufs=0, dma_split=True, t2c_act=False,
             reps=1):
    """Build the SPMD single-core program (same program on all cores).

    reps > 1 repeats the whole kernel in sequence (one TileContext per
    rep, separated by the context-exit drain+barrier) -- used only for
    slope-based HW timing, where per-call dispatch overhead cancels.
    """
    h = s_per // P
    hd_total = sum(g[0] for g in groups)
    hg_total = sum(g[1] for g in groups)
    assert hd_total + hg_total == h

    nc = bacc.Bacc("TRN2", target_bir_lowering=False, debug=False,
                   num_devices=N_CORES)
    x = nc.dram_tensor("x", [s_per], F32, kind="ExternalInput")
    out = nc.dram_tensor("out", [s_per, d], F32, kind="ExternalOutput")

    x2d = x.rearrange("(p h) -> p h", p=P)          # [128, h]
    out3 = out.rearrange("(p h) d -> p h d", p=P)   # [128, h, d]

    # h-slot ranges: DVE owns h [0, hd_total), Pool owns [hd_total, h)
    sub_tiles = []  # (engine_name, h_start, nh)
    pos_d, pos_g = 0, hd_total
    for (nd, ng) in groups:
        if nd:
            sub_tiles.append(("vector", pos_d, nd))
            pos_d += nd
        if ng:
            sub_tiles.append(("gpsimd", pos_g, ng))
            pos_g += ng
    seed_splits = (("vector", 0, hd_total), ("gpsimd", hd_total, hg_total))

    for _rep in range(reps):
        with tile.TileContext(nc) as tc:
            with (
                tc.tile_pool(name="xp", bufs=1) as xpool,
                tc.tile_pool(name="seed", bufs=1) as seedpool,
                tc.tile_pool(name="grp", bufs=gbufs or 6) as gpool,
                tc.tile_pool(name="small", bufs=8) as spool,
            ):
                _emit_body(nc, tc, (xpool, seedpool, gpool, spool), s_per, d,
                           base, sub_tiles, seed_splits, max_seq_len, x2d,
                           out3, dma_split, t2c_act)

    nc.compile()
    return nc


_CACHED_NC = None


def kernel(input_ids, max_seq_len, d_model):
    """Full-input entry point: shards batch rows across the 8 cores."""
    global _CACHED_NC
    input_ids = np.ascontiguousarray(np.asarray(input_ids, dtype=np.float32))
    assert input_ids.shape == (B, S) and int(max_seq_len) == MAX_SEQ_LEN \
        and int(d_model) == D
    if _CACHED_NC is None:
        _CACHED_NC = build_nc()
    in_maps = [{"x": input_ids[c]} for c in range(N_CORES)]
    res = run_bass_kernel_spmd(_CACHED_NC, in_maps,
                               core_ids=list(range(N_CORES)))
    return np.stack([res.results[c]["out"] for c in range(N_CORES)], axis=0)

